# revision 13
# baseline (speedup 1.0000x reference)
"""Trainium2 kernel for nn_DigitConvolutionalModel (dense_cnn).

Model: x[B,784] -> 3x3 valid conv (single channel) -> flatten[676]
       -> Linear(676,200) + ReLU -> Linear(200,10).

The conv is linear, so it is folded into the first Linear on the host:
  flat = x @ C  (C [784,676] sparse conv matrix)
  h1   = relu(flat @ W1.T + b1) = relu(x @ (C @ W1.T) + b1)
so the device computes a plain 784 -> 200 -> 10 MLP. Pure data
parallelism: batch 32768 is split into 8 shards of 4096, one per core;
weights are replicated. Each core receives x pre-transposed ([784,4096],
pixel on the partition/contraction axis) so both matmuls need no
on-device transpose:
  FC1: h1T[200,b] = W1eff[784,200].T @ xT[784,b]   (lhsT = W1eff)
  FC2: outT[10,b] = W2T[200,10].T  @ h1T[200,b]    (lhsT = W2.T)
"""

import os
import numpy as np
from contextlib import ExitStack

import concourse.bass as bass
import concourse.bacc as bacc
import concourse.mybir as mybir
import concourse.tile as tile
from concourse.bass_utils import run_bass_kernel_spmd

import ml_dtypes

N_CORES = 8
B = 32768
BS = B // N_CORES          # 4096 rows per core
IMG = 28
KSZ = 3
OUTW = IMG - KSZ + 1       # 26
NPIX = IMG * IMG           # 784
HID = 200
NCLS = 10

P = 128                    # SBUF partitions
FD = 512                   # matmul free dim = one fp32 PSUM bank
NK = 7                     # contraction tiles over 784 = 6*128 + 16
KT = [P] * 6 + [NPIX - 6 * P]
MT = [P, HID - P]          # hid output tiles: 128 + 72
NHALF = 2                  # batch halves per core (PSUM: 2m x 4n = 8 banks)
HB = BS // NHALF           # 2048
NT = HB // FD              # 4 n-tiles of 512 per half

_cache: dict = {}


def _ensure_axon_hooks():
    """Provide antenv.axon_hooks if the image lacks it.

    bass_utils' trace path does `from antenv.axon_hooks import
    get_axon_ntff_profile_hook`; on images without that module the import
    crashes instead of degrading. Register a minimal equivalent that drives
    NTFF profiling via the documented C ABI of the loaded axon PJRT plugin
    (axon_start_nrt_profile / axon_stop_nrt_profile), or returns None so
    bass_utils skips tracing gracefully.
    """
    try:
        import antenv.axon_hooks  # noqa: F401

        return
    except ImportError:
        pass
    import sys
    import types
    import ctypes
    import contextlib

    try:
        import antenv
    except ImportError:
        antenv = types.ModuleType("antenv")
        sys.modules["antenv"] = antenv

    mod = types.ModuleType("antenv.axon_hooks")
    state = {"hook": None, "built": False}

    def _build():
        so_path = None
        try:
            with open("/proc/self/maps") as f:
                for line in f:
                    if "libaxon_pjrt.so" in line:
                        so_path = line.split()[-1]
                        break
        except OSError:
            return None
        if so_path is None:
            return None
        lib = ctypes.CDLL(so_path)
        if not hasattr(lib, "axon_start_nrt_profile"):
            return None
        lib.axon_start_nrt_profile.argtypes = [
            ctypes.POINTER(ctypes.c_int64),
            ctypes.c_size_t,
        ]
        lib.axon_start_nrt_profile.restype = ctypes.c_int64
        lib.axon_stop_nrt_profile.argtypes = [ctypes.c_char_p]
        lib.axon_stop_nrt_profile.restype = ctypes.c_int64

        @contextlib.contextmanager
        def _hook(output_dir, device_ids):
            import jax

            jax.devices()
            if device_ids:
                ids = (ctypes.c_int64 * len(device_ids))(*device_ids)
                rc = lib.axon_start_nrt_profile(ids, len(device_ids))
            else:
                rc = lib.axon_start_nrt_profile(None, 0)
            if rc != 0:
                raise RuntimeError(f"axon_start_nrt_profile rc={rc}")
            try:
                yield
            finally:
                n = lib.axon_stop_nrt_profile(str(output_dir).encode())
                if n <= 0:
                    print(f"ntff profile: rc={n} (no profile written)")

        return _hook

    def get_axon_ntff_profile_hook():
        if not state["built"]:
            state["hook"] = _build()
            state["built"] = True
        return state["hook"]

    def set_axon_ntff_profile_hook(hook):
        state["hook"] = hook
        state["built"] = True

    mod.get_axon_ntff_profile_hook = get_axon_ntff_profile_hook
    mod.set_axon_ntff_profile_hook = set_axon_ntff_profile_hook
    sys.modules["antenv.axon_hooks"] = mod
    antenv.axon_hooks = mod


def _dtypes():
    if os.environ.get("KERNEL_FP32"):
        return mybir.dt.float32, np.float32
    return mybir.dt.bfloat16, ml_dtypes.bfloat16


def _build_nc():
    mm_dt, _ = _dtypes()
    f32 = mybir.dt.float32
    # Bacc (not plain Bass): its compile() pass splits multi-sem waits into
    # standalone EventSemaphore instructions — the TPB ISA allows only one
    # embedded wait per instruction.
    nc = bacc.Bacc(
        "TRN2",
        target_bir_lowering=False,
        debug=False,
        num_devices=N_CORES,
    )

    xT = nc.dram_tensor("xT", [NPIX, BS], mm_dt, kind="ExternalInput")
    w1 = nc.dram_tensor("w1t", [P, NK * HID], mm_dt, kind="ExternalInput")
    w2 = nc.dram_tensor("w2t", [P, 2 * NCLS], mm_dt, kind="ExternalInput")
    b1 = nc.dram_tensor("b1t", [P, 2], f32, kind="ExternalInput")
    b2 = nc.dram_tensor("b2t", [NCLS, 1], f32, kind="ExternalInput")
    outT = nc.dram_tensor("outT", [NCLS, BS], f32, kind="ExternalOutput")

    with ExitStack() as ctx:
        tc = ctx.enter_context(tile.TileContext(nc))
        const = ctx.enter_context(tc.tile_pool(name="const", bufs=1))
        xp = ctx.enter_context(tc.tile_pool(name="xp", bufs=NHALF * NK))
        h1p = ctx.enter_context(tc.tile_pool(name="h1p", bufs=2))
        op = ctx.enter_context(tc.tile_pool(name="op", bufs=NHALF * NT))
        pp = ctx.enter_context(tc.tile_pool(name="pp", bufs=8, space="PSUM"))

        w1s = const.tile([P, NK * HID], mm_dt)
        w2s = const.tile([P, 2 * NCLS], mm_dt)
        b1s = const.tile([P, 2], f32)
        b2s = const.tile([NCLS, 1], f32)
        nc.sync.dma_start(w1s[:], w1[:])
        nc.sync.dma_start(w2s[:], w2[:])
        nc.sync.dma_start(b1s[:], b1[:])
        nc.sync.dma_start(b2s[:], b2[:])

        h1tiles = []
        for h in range(NHALF):
            c0 = h * HB
            ps = [
                [
                    pp.tile([MT[m], FD], f32, tag="bank", name=f"ps_{h}_{m}_{n}")
                    for n in range(NT)
                ]
                for m in range(2)
            ]
            for k in range(NK):
                kt = KT[k]
                xt = xp.tile([P, HB], mm_dt, tag="xt")
                nc.sync.dma_start(xt[:kt, :], xT[k * P : k * P + kt, c0 : c0 + HB])
                for m in range(2):
                    lhsT = w1s[0:kt, k * HID + m * P : k * HID + m * P + MT[m]]
                    for n in range(NT):
                        nc.tensor.matmul(
                            ps[m][n][:],
                            lhsT,
                            xt[0:kt, n * FD : (n + 1) * FD],
                            start=(k == 0),
                            stop=(k == NK - 1),
                        )
            h1 = [
                h1p.tile([MT[0], HB], mm_dt, tag="h1a", name=f"h1a_{h}"),
                h1p.tile([MT[1], HB], mm_dt, tag="h1b", name=f"h1b_{h}"),
            ]
            for m in range(2):
                for n in range(NT):
                    nc.scalar.activation(
                        h1[m][:, n * FD : (n + 1) * FD],
                        ps[m][n][:],
                        mybir.ActivationFunctionType.Relu,
                        bias=b1s[0 : MT[m], m : m + 1],
                    )
            h1tiles.append(h1)

        # FC2 emitted after all FC1 matmuls: PE stays dense through FC1,
        # FC2 runs at the tail when h1 has long been drained.
        for h in range(NHALF):
            c0 = h * HB
            h1 = h1tiles[h]
            for n in range(NT):
                ps2 = pp.tile([NCLS, FD], f32, tag="bank", name=f"ps2_{h}_{n}")
                for m in range(2):
                    nc.tensor.matmul(
                        ps2[:],
                        w2s[0 : MT[m], m * NCLS : (m + 1) * NCLS],
                        h1[m][:, n * FD : (n + 1) * FD],
                        start=(m == 0),
                        stop=(m == 1),
                    )
                ot = op.tile([NCLS, FD], f32, tag="ot", name=f"ot_{h}_{n}")
                nc.vector.tensor_scalar_add(ot[:], ps2[:], b2s[:])
                nc.sync.dma_start(
                    outT[:, c0 + n * FD : c0 + (n + 1) * FD], ot[:]
                )

    nc.compile()
    nc.finalize()
    return nc


def _fold_weights(conv_w, W1):
    """W1eff[784,200] such that x @ W1eff == conv2d_valid(x, conv_w).flat @ W1.T"""
    W1r = W1.reshape(HID, OUTW, OUTW).transpose(1, 2, 0)  # [26,26,200]
    w1e = np.zeros((IMG, IMG, HID), np.float32)
    for di in range(KSZ):
        for dj in range(KSZ):
            w1e[di : di + OUTW, dj : dj + OUTW, :] += conv_w[di, dj] * W1r
    return w1e.reshape(NPIX, HID)


def _prepare_maps(x, conv_w, W1, b1, W2, b2):
    _, np_dt = _dtypes()
    x = np.asarray(x, np.float32)
    conv_w = np.asarray(conv_w, np.float32)
    W1 = np.asarray(W1, np.float32)
    b1 = np.asarray(b1, np.float32)
    W2 = np.asarray(W2, np.float32)
    b2 = np.asarray(b2, np.float32)

    w1e = _fold_weights(conv_w, W1)
    w1t = np.zeros((P, NK * HID), np_dt)
    for k in range(NK):
        kt = KT[k]
        w1t[:kt, k * HID : (k + 1) * HID] = w1e[k * P : k * P + kt, :].astype(np_dt)
    W2T = W2.T  # [200, 10]
    w2t = np.zeros((P, 2 * NCLS), np_dt)
    w2t[: MT[0], 0:NCLS] = W2T[:P].astype(np_dt)
    w2t[: MT[1], NCLS : 2 * NCLS] = W2T[P:].astype(np_dt)
    b1t = np.zeros((P, 2), np.float32)
    b1t[: MT[0], 0] = b1[:P]
    b1t[: MT[1], 1] = b1[P:]
    b2t = b2.reshape(NCLS, 1)

    xs = x.reshape(N_CORES, BS, NPIX)
    return [
        {
            "xT": xs[i].T.astype(np_dt),
            "w1t": w1t,
            "w2t": w2t,
            "b1t": b1t,
            "b2t": b2t,
        }
        for i in range(N_CORES)
    ]


def _run(inputs, trace=False):
    _ensure_axon_hooks()
    key = ("nc", bool(os.environ.get("KERNEL_FP32")))
    if key not in _cache:
        _cache[key] = _build_nc()
    nc = _cache[key]
    in_maps = _prepare_maps(**inputs)
    res = run_bass_kernel_spmd(nc, in_maps, list(range(N_CORES)), trace=trace)
    out = np.concatenate([r["outT"].T for r in res.results], axis=0)
    return out, res


def kernel(**inputs):
    out, _ = _run(inputs, trace=False)
    return out
